# revision 28
# baseline (speedup 1.0000x reference)
"""AEG-Conv2d Trainium2 kernel (8 NeuronCores, data-parallel over batch).

Math: the reference's 9-step scan  r <- (r+x)*y / (r+y)*x  (parity of i+j+k)
unrolls to  r = sum_k a_k * prod_{j>=k} m_j, which factors per output-pixel
parity into  out[n,oc,px] = sum_{t=0..8} sum_ic U_t[n,ic,px] * V_t[oc,ic]:
a 288-deep contraction where U_t are products of shifted input patches
(computed on-chip) and V_t are products of weight taps (computed on host).

Per-core layout: partition p = rg*32 + ic (rg = 4 row-groups of 16 output
rows, with 18-row halo of the 66-wide padded image in the free dim).
Checkerboard pixel sets are addressed with diagonal access patterns
[(132,8),(67|65,2),(2,32)] giving dense 512-px per-parity feature tiles.
Matmuls are K=32 (ic) row-tiled at partition base 32*rg, M=64 (oc),
accumulating 9 taps into PSUM bank rg ([0:64]=even-parity out, [64:128]=odd).
"""

import numpy as np

IC, OC, H, W = 32, 64, 64, 64
N = 8
RG = 4           # row groups per core
PADW, PADR = 66, 18
XFREE = PADR * PADW          # 1188
VFREE = 9 * 2 * OC           # 1152
OUTFREE = RG * 512           # 2048

MM_DTYPE = "bfloat16"        # matmul input dtype (1 PE cycle/row; fp32 would be 4)
N_WARM = 7                   # dummy full-mode matmuls to warm the PE clock

# feature chains: (t, factor_tap, source_t); U_t = P_factor ⊙ U_source.
# sources 7,8 (even) and 8 (odd) are raw f32 patch views into xp.
# Split across engines so DVE and GPSIMD produce features concurrently
# (GPSIMD TT is ~2x slower per op, so it gets the short odd-parity spine).
_DVE_OPS = [  # (parity, t, factor, src)
    (0, 5, 5, 7), (0, 6, 6, 7), (0, 3, 3, 5), (0, 4, 4, 5), (1, 5, 5, 6),
    (0, 1, 1, 3), (0, 2, 2, 3), (1, 3, 3, 4), (0, 0, 0, 1), (1, 1, 1, 2),
]
_GP_OPS = [
    (1, 7, 7, 8), (1, 6, 6, 8), (1, 4, 4, 6), (1, 2, 2, 4), (1, 0, 0, 2),
]
_ROOTS = [(0, 7), (0, 8), (1, 8)]       # bf16 cast-copies on ScalarE (MM rhs only)
_MM_ORDER = {0: [7, 8, 5, 6, 3, 4, 1, 2, 0], 1: [8, 7, 6, 5, 4, 3, 2, 1, 0]}


def _bass_modules():
    import sys
    try:
        import concourse.bass as bass
    except ImportError:
        sys.path.insert(0, "/opt/trn_rl_repo")
        import concourse.bass as bass
    import concourse.mybir as mybir
    import concourse.tile as tile
    from concourse import bass_utils
    return bass, mybir, tile, bass_utils


def _hoist_input_dmas(nc, mybir, names):
    """Move the input-load DMACopy triggers (no waits, SP engine) from the
    body block into the prologue block, before SP enters the all-engine
    barrier — the loads then overlap the other engines' preamble."""
    f = nc.m.functions[0]
    blocks = list(f.blocks)
    pro, body = blocks[0], blocks[1]
    moved = []
    bil = body.instructions
    i = 0
    while i < len(bil):
        inst = bil[i]
        si = inst.sync_info
        if (type(inst).__name__ == "InstDMACopy"
                and (si is None or not list(si.on_wait))):
            moved.append(inst)    # input loads are the only wait-free DMAs
            del bil[i]
            continue
        i += 1
    pil = pro.instructions
    # insert before the SP barrier drain (first SP InstDrain in prologue)
    idx = 0
    for j, inst in enumerate(pil):
        if type(inst).__name__ == "InstDrain" and inst.engine == mybir.EngineType.SP:
            idx = j
            break
    for k, inst in enumerate(moved):
        pil.insert(idx + k, inst)


def _split_multi_waits(nc, mybir, limit=1):
    """walrus codegen in this toolchain allows only one sync-wait command per
    engine instruction; hoist surplus waits into standalone InstEventSemaphore
    instructions inserted just before, on the same engine queue."""
    n = 0
    for f in nc.m.functions:
        for b in f.blocks:
            il = b.instructions
            i = 0
            while i < len(il):
                inst = il[i]
                si = inst.sync_info
                if si is not None and len(si.on_wait) > limit:
                    waits = list(si.on_wait)
                    for w in waits[:-limit]:
                        ev = mybir.InstEventSemaphore(
                            name=f"wsplit_{n}", ins=[], outs=[])
                        n += 1
                        ev.engine = inst.engine
                        ev.sync_info = mybir.SyncInfo(on_wait=[w], on_update=[])
                        il.insert(i, ev)
                        i += 1
                    inst.sync_info = mybir.SyncInfo(
                        on_wait=waits[-limit:], on_update=list(si.on_update))
                i += 1


def build_nc(split_waits=True):
    bass, mybir, tile, _ = _bass_modules()
    F32 = mybir.dt.float32
    mmdt = getattr(mybir.dt, MM_DTYPE)
    nc = bass.Bass()
    x_in = nc.declare_dram_parameter("xpad", [128, XFREE], F32, isOutput=False)
    v_in = nc.declare_dram_parameter("vtab", [128, VFREE], mmdt, isOutput=False)
    out_ext = nc.declare_dram_parameter("out", [128, OUTFREE], F32, isOutput=True)
    BF16 = mybir.dt.bfloat16

    def diag(xp_ap, k, parity, rg=None):
        # checkerboard view: free dims (s,e,m) -> padded px (2s+e+dh, 2m+e+dw)
        # for even parity, (2s+e+dh, 2m+(1-e)+dw) for odd.
        dh, dw = divmod(k, 3)
        base = PADW * dh + dw + (1 if parity else 0)
        estep = PADW + 1 if parity == 0 else PADW - 1
        v = xp_ap.copy()
        pstep = tuple(list(v.ap)[0])[0]
        poff = 0 if rg is None else 32 * rg * pstep
        cnt = 128 if rg is None else 32
        v.ap = mybir.VecI64Pair([(pstep, cnt), (2 * PADW, 8), (estep, 2), (2, 32)])
        v.offset = v.offset + poff + base
        return v

    with tile.TileContext(nc) as tc:
        with tc.tile_pool(name="sb", bufs=1) as pool, \
             tc.tile_pool(name="ps", bufs=1, space="PSUM") as pp:
            xp = pool.tile([128, XFREE], F32, tag="xp")
            nc.sync.dma_start(xp[:], x_in[:])
            vt = pool.tile([128, VFREE], mmdt, tag="vt")
            nc.sync.dma_start(vt[:], v_in[:])
            outb = pool.tile([128, OUTFREE], F32, tag="outb")

            feats = {}

            def newfeat(par, t):
                ft = pool.tile([128, 512], mmdt, tag=f"f{par}_{t}", name=f"f{par}_{t}")
                feats[(par, t)] = ft
                return ft[:].rearrange("p (s e m) -> p s e m", e=2, m=32)

            rootset = {(p, t) for p, t in _ROOTS}

            def src_ap(par, src):
                if (par, src) in rootset:
                    return diag(xp[:], src, par)   # raw f32 view, no copy dep
                return feats[(par, src)][:].rearrange(
                    "p (s e m) -> p s e m", e=2, m=32)

            # roots: bf16 dense copies for MM rhs (ScalarE, off the DVE path)
            for par, t in _ROOTS:
                nc.scalar.copy(newfeat(par, t), diag(xp[:], t, par))
            # chain muls, interleaved across DVE and GPSIMD
            qs = {0: list(_DVE_OPS), 1: list(_GP_OPS)}
            engines = {0: nc.vector, 1: nc.gpsimd}
            while qs[0] or qs[1]:
                for q in (0, 1, 0):   # 2 DVE ops per GP op (DVE is ~2x faster)
                    if qs[q]:
                        par, t, kf, src = qs[q].pop(0)
                        in1 = src_ap(par, src)
                        engines[q].tensor_mul(
                            newfeat(par, t), diag(xp[:], kf, par), in1)

            psums = [pp.tile([128, 512], F32, tag=f"acc{rg}", name=f"acc{rg}")
                     for rg in range(RG)]
            # PE warm-up: tiled-mode (K=32) matmuls don't engage the HAM
            # clock gate, leaving the PE at 1.2 GHz. Run full-width dummy
            # matmuls on the vt tile while DVE/GPSIMD produce features, so
            # the real accumulations run at 2.4 GHz.
            wps = pp.tile([128, 512], F32, tag="warm", name="warm")
            for w in range(N_WARM):
                nc.tensor.matmul(wps[:], lhsT=vt[:, 0:128], rhs=vt[:, 0:512],
                                 start=True, stop=True, skip_group_check=True)
            for i in range(9):
                # keep the HAM clock-gate open: tiled MMs don't count as
                # PE-busy, so feed it one full-mode MM per round
                nc.tensor.matmul(wps[:], lhsT=vt[:, 0:128], rhs=vt[:, 0:512],
                                 start=True, stop=True, skip_group_check=True)
                for par in (0, 1):
                    t = _MM_ORDER[par][i]
                    for rg in range(RG):
                        if (par, t) in feats:
                            rhs = feats[(par, t)][32 * rg:32 * (rg + 1), :]
                        else:
                            rhs = diag(xp[:], t, par, rg=rg)
                        lhsT = vt[32 * rg:32 * (rg + 1),
                                  t * 128 + 64 * par: t * 128 + 64 * par + 64]
                        nc.tensor.matmul(
                            psums[rg][64 * par:64 * par + 64, :],
                            lhsT=lhsT, rhs=rhs,
                            start=(i == 0), stop=(i == 8),
                            skip_group_check=True,
                            tile_position=(32 * rg, 64 * par))
            # evacuate PSUM on both ScalarE and VectorE; stream each bank's
            # result to DRAM as soon as it is copied
            for rg in range(RG):
                dst = outb[:, 512 * rg:512 * (rg + 1)]
                if rg % 2 == 0:
                    nc.scalar.copy(dst, psums[rg][:])
                else:
                    nc.vector.tensor_copy(dst, psums[rg][:])
                nc.sync.dma_start(
                    out_ext[:, 512 * rg:512 * (rg + 1)], dst)
    if split_waits:
        _hoist_input_dmas(nc, mybir, ("xpad", "vtab"))
        _split_multi_waits(nc, mybir)
    return nc


def host_inputs(x, weight):
    y = weight.reshape(OC, IC, 9).transpose(2, 0, 1).astype(np.float64)
    V = np.empty_like(y)
    V[8] = y[8]; V[6] = y[6] * V[8]; V[4] = y[4] * V[6]; V[2] = y[2] * V[4]
    V[0] = y[0] * V[2]; V[1] = y[1] * V[2]; V[3] = y[3] * V[4]; V[5] = y[5] * V[6]
    V[7] = y[7] * V[8]
    Vo = np.empty_like(y)
    Vo[7] = y[7]; Vo[5] = y[5] * Vo[7]; Vo[3] = y[3] * Vo[5]; Vo[1] = y[1] * Vo[3]
    Vo[0] = y[0] * Vo[1]; Vo[2] = y[2] * Vo[3]; Vo[4] = y[4] * Vo[5]; Vo[6] = y[6] * Vo[7]
    Vo[8] = y[8]
    import ml_dtypes
    vt = np.stack([V, Vo], 1)                                   # (9, 2, OC, IC)
    vflat = vt.transpose(3, 0, 1, 2).reshape(IC, VFREE)
    vtab = np.ascontiguousarray(
        np.tile(vflat, (RG, 1)).astype(ml_dtypes.bfloat16))     # (128, 1152)

    xpads = []
    for i in range(x.shape[0]):
        xpi = np.pad(x[i], ((0, 0), (1, 1), (1, 1)))
        parts = np.stack([xpi[:, 16 * rg:16 * rg + PADR, :] for rg in range(RG)], 0)
        xpads.append(np.ascontiguousarray(parts.reshape(128, XFREE), dtype=np.float32))
    return xpads, vtab


_RGI = np.arange(RG)[:, None, None, None]
_SI = np.arange(8)[None, :, None, None]
_EI = np.arange(2)[None, None, :, None]
_MI = np.arange(32)[None, None, None, :]
_ROWS = 16 * _RGI + 2 * _SI + _EI
_COLE = 2 * _MI + _EI
_COLO = 2 * _MI + 1 - _EI


def decode_out(out):
    o = out.reshape(2, OC, RG, 8, 2, 32)
    full = np.empty((OC, H, W), np.float32)
    full[:, _ROWS, _COLE] = o[0]
    full[:, _ROWS, _COLO] = o[1]
    return full


def kernel(x, weight):
    _, _, _, bass_utils = _bass_modules()
    x = np.ascontiguousarray(np.asarray(x), dtype=np.float32)
    weight = np.ascontiguousarray(np.asarray(weight), dtype=np.float32)
    xpads, vtab = host_inputs(x, weight)
    nc = build_nc()
    in_maps = [{"xpad": xpads[i], "vtab": vtab} for i in range(N)]
    res = bass_utils.run_bass_kernel_spmd(nc, in_maps, core_ids=list(range(N)))
    return np.stack([decode_out(res.results[i]["out"]) for i in range(N)], 0)


# revision 36
# speedup vs baseline: 1.0775x; 1.0775x over previous
"""AEG-Conv2d Trainium2 kernel (8 NeuronCores, data-parallel over batch).

Math: the reference's 9-step scan  r <- (r+x)*y / (r+y)*x  (parity of i+j+k)
unrolls to  r = sum_k a_k * prod_{j>=k} m_j, which factors per output-pixel
parity into  out[n,oc,px] = sum_{t=0..8} sum_ic U_t[n,ic,px] * V_t[oc,ic]:
a 288-deep contraction where U_t are products of shifted input patches
(computed on-chip) and V_t are products of weight taps (computed on host).

Per-core layout: partition p = rg*32 + ic (rg = 4 row-groups of 16 output
rows, with 18-row halo of the 66-wide padded image in the free dim).
Checkerboard pixel sets are addressed with diagonal access patterns
[(132,8),(67|65,2),(2,32)] giving dense 512-px per-parity feature tiles.
Matmuls are K=32 (ic) row-tiled at partition base 32*rg, M=64 (oc),
accumulating 9 taps into PSUM bank rg ([0:64]=even-parity out, [64:128]=odd).
"""

import numpy as np

IC, OC, H, W = 32, 64, 64, 64
N = 8
RG = 4           # row groups per core
PADW, PADR = 66, 18
PLW = 34         # parity-plane row width (33 entries + 1 pad)
PLSZ = PADR * PLW            # 612 per plane
XFREE = 2 * PLSZ             # 1224
VFREE = 9 * 2 * OC           # 1152
OUTFREE = RG * 512           # 2048

MM_DTYPE = "bfloat16"        # matmul input dtype (1 PE cycle/row; fp32 would be 4)
N_WARM = 7                   # dummy full-mode matmuls to warm the PE clock

# feature chains: (t, factor_tap, source_t); U_t = P_factor ⊙ U_source.
# sources 7,8 (even) and 8 (odd) are raw f32 patch views into xp.
# Split across engines so DVE and GPSIMD produce features concurrently
# (GPSIMD TT is ~2x slower per op, so it gets the short odd-parity spine).
_DVE_OPS = [  # (parity, t, factor, src)
    (0, 5, 5, 7), (0, 6, 6, 7), (0, 3, 3, 5), (0, 4, 4, 5), (1, 5, 5, 6),
    (0, 1, 1, 3), (0, 2, 2, 3), (1, 3, 3, 4), (0, 0, 0, 1), (1, 1, 1, 2),
    (1, 0, 0, 2),
]
_GP_OPS = [
    (1, 7, 7, 8), (1, 6, 6, 8), (1, 4, 4, 6), (1, 2, 2, 4),
]
_ROOTS = [(0, 7), (0, 8), (1, 8)]       # raw plane views (no materialization)
_MM_ORDER = {0: [7, 8, 5, 6, 3, 4, 1, 2, 0], 1: [8, 7, 6, 5, 4, 3, 2, 1, 0]}


def _bass_modules():
    import sys
    try:
        import concourse.bass as bass
    except ImportError:
        sys.path.insert(0, "/opt/trn_rl_repo")
        import concourse.bass as bass
    import concourse.mybir as mybir
    import concourse.tile as tile
    from concourse import bass_utils
    return bass, mybir, tile, bass_utils


def _hoist_input_dmas(nc, mybir, names):
    """Move the input-load DMACopy triggers (no waits, SP engine) from the
    body block into the prologue block, before SP enters the all-engine
    barrier — the loads then overlap the other engines' preamble."""
    f = nc.m.functions[0]
    blocks = list(f.blocks)
    pro, body = blocks[0], blocks[1]
    moved = []
    bil = body.instructions
    i = 0
    while i < len(bil):
        inst = bil[i]
        si = inst.sync_info
        if (type(inst).__name__ == "InstDMACopy"
                and (si is None or not list(si.on_wait))):
            moved.append(inst)    # input loads are the only wait-free DMAs
            del bil[i]
            continue
        i += 1
    pil = pro.instructions
    # insert before the SP barrier drain (first SP InstDrain in prologue)
    idx = 0
    for j, inst in enumerate(pil):
        if type(inst).__name__ == "InstDrain" and inst.engine == mybir.EngineType.SP:
            idx = j
            break
    for k, inst in enumerate(moved):
        pil.insert(idx + k, inst)


def _split_multi_waits(nc, mybir, limit=1):
    """walrus codegen in this toolchain allows only one sync-wait command per
    engine instruction; hoist surplus waits into standalone InstEventSemaphore
    instructions inserted just before, on the same engine queue."""
    n = 0
    for f in nc.m.functions:
        for b in f.blocks:
            il = b.instructions
            i = 0
            while i < len(il):
                inst = il[i]
                si = inst.sync_info
                if si is not None and len(si.on_wait) > limit:
                    waits = list(si.on_wait)
                    for w in waits[:-limit]:
                        ev = mybir.InstEventSemaphore(
                            name=f"wsplit_{n}", ins=[], outs=[])
                        n += 1
                        ev.engine = inst.engine
                        ev.sync_info = mybir.SyncInfo(on_wait=[w], on_update=[])
                        il.insert(i, ev)
                        i += 1
                    inst.sync_info = mybir.SyncInfo(
                        on_wait=waits[-limit:], on_update=list(si.on_update))
                i += 1


def build_nc(split_waits=True):
    bass, mybir, tile, _ = _bass_modules()
    F32 = mybir.dt.float32
    mmdt = getattr(mybir.dt, MM_DTYPE)
    nc = bass.Bass()
    x_in = nc.declare_dram_parameter("xpad", [128, XFREE], mybir.dt.bfloat16,
                                     isOutput=False)
    v_in = nc.declare_dram_parameter("vtab", [128, VFREE], mmdt, isOutput=False)
    out_ext = nc.declare_dram_parameter("out", [128, OUTFREE], F32, isOutput=True)
    BF16 = mybir.dt.bfloat16

    def diag(xp_ap, k, par, rg=None):
        """Patch-tap view over the parity-plane xp layout.
        Free dims (s,e,m) address output px (row 2s+e, col 2m+((par+e)%2));
        tap k=(dh,dw) reads plane (par+dh+dw)%2 densely (inner step 1)."""
        dh, dw = divmod(k, 3)
        qk = (par + dh + dw) % 2
        def delta(e):
            return ((par + e) % 2 + dw - (e + dh + qk) % 2) // 2
        base = qk * PLSZ + dh * PLW + delta(0)
        estep = PLW + delta(1) - delta(0)
        v = xp_ap.copy()
        pstep = tuple(list(v.ap)[0])[0]
        poff = 0 if rg is None else 32 * rg * pstep
        cnt = 128 if rg is None else 32
        v.ap = mybir.VecI64Pair([(pstep, cnt), (2 * PLW, 8), (estep, 2), (1, 32)])
        v.offset = v.offset + poff + base
        return v

    with tile.TileContext(nc) as tc:
        with tc.tile_pool(name="sb", bufs=1) as pool, \
             tc.tile_pool(name="ps", bufs=1, space="PSUM") as pp:
            xp = pool.tile([128, XFREE], BF16, tag="xp")
            nc.sync.dma_start(xp[:], x_in[:])
            vt = pool.tile([128, VFREE], mmdt, tag="vt")
            nc.sync.dma_start(vt[:], v_in[:])
            outb = pool.tile([128, OUTFREE], F32, tag="outb")

            feats = {}

            def newfeat(par, t):
                ft = pool.tile([128, 512], mmdt, tag=f"f{par}_{t}", name=f"f{par}_{t}")
                feats[(par, t)] = ft
                return ft[:].rearrange("p (s e m) -> p s e m", e=2, m=32)

            rootset = {(p, t) for p, t in _ROOTS}

            def src_ap(par, src):
                if (par, src) in rootset:
                    return diag(xp[:], src, par)   # raw plane view, no copy
                return feats[(par, src)][:].rearrange(
                    "p (s e m) -> p s e m", e=2, m=32)

            # chain muls, interleaved across DVE and GPSIMD
            qs = {0: list(_DVE_OPS), 1: list(_GP_OPS)}
            engines = {0: nc.vector, 1: nc.gpsimd}
            while qs[0] or qs[1]:
                for q in (0, 0, 1):   # ~3 DVE ops per GP op
                    if qs[q]:
                        par, t, kf, src = qs[q].pop(0)
                        in1 = src_ap(par, src)
                        engines[q].tensor_mul(
                            newfeat(par, t), diag(xp[:], kf, par), in1)

            psums = [pp.tile([128, 512], F32, tag=f"acc{rg}", name=f"acc{rg}")
                     for rg in range(RG)]
            # PE warm-up: tiled-mode (K=32) matmuls don't engage the HAM
            # clock gate, leaving the PE at 1.2 GHz. Run full-width dummy
            # matmuls on the vt tile while DVE/GPSIMD produce features, so
            # the real accumulations run at 2.4 GHz.
            wps = pp.tile([128, 512], F32, tag="warm", name="warm")
            for w in range(N_WARM):
                nc.tensor.matmul(wps[:], lhsT=vt[:, 0:128], rhs=vt[:, 0:512],
                                 start=True, stop=True, skip_group_check=True)
            for i in range(9):
                # keep the HAM clock-gate open: tiled MMs don't count as
                # PE-busy, so feed it one full-mode MM per round
                nc.tensor.matmul(wps[:], lhsT=vt[:, 0:128], rhs=vt[:, 0:512],
                                 start=True, stop=True, skip_group_check=True)
                for par in (0, 1):
                    t = _MM_ORDER[par][i]
                    for rg in range(RG):
                        if (par, t) in feats:
                            rhs = feats[(par, t)][32 * rg:32 * (rg + 1), :]
                        else:
                            rhs = diag(xp[:], t, par, rg=rg)  # raw plane view
                        lhsT = vt[32 * rg:32 * (rg + 1),
                                  t * 128 + 64 * par: t * 128 + 64 * par + 64]
                        nc.tensor.matmul(
                            psums[rg][64 * par:64 * par + 64, :],
                            lhsT=lhsT, rhs=rhs,
                            start=(i == 0), stop=(i == 8),
                            skip_group_check=True,
                            tile_position=(32 * rg, 64 * par))
            # evacuate PSUM on both ScalarE and VectorE; stream each bank's
            # result to DRAM as soon as it is copied
            for rg in range(RG):
                dst = outb[:, 512 * rg:512 * (rg + 1)]
                if rg % 2 == 0:
                    nc.scalar.copy(dst, psums[rg][:])
                else:
                    nc.vector.tensor_copy(dst, psums[rg][:])
                nc.sync.dma_start(
                    out_ext[:, 512 * rg:512 * (rg + 1)], dst)
    if split_waits:
        _hoist_input_dmas(nc, mybir, ("xpad", "vtab"))
        _split_multi_waits(nc, mybir)
    return nc


def host_inputs(x, weight):
    y = weight.reshape(OC, IC, 9).transpose(2, 0, 1).astype(np.float64)
    V = np.empty_like(y)
    V[8] = y[8]; V[6] = y[6] * V[8]; V[4] = y[4] * V[6]; V[2] = y[2] * V[4]
    V[0] = y[0] * V[2]; V[1] = y[1] * V[2]; V[3] = y[3] * V[4]; V[5] = y[5] * V[6]
    V[7] = y[7] * V[8]
    Vo = np.empty_like(y)
    Vo[7] = y[7]; Vo[5] = y[5] * Vo[7]; Vo[3] = y[3] * Vo[5]; Vo[1] = y[1] * Vo[3]
    Vo[0] = y[0] * Vo[1]; Vo[2] = y[2] * Vo[3]; Vo[4] = y[4] * Vo[5]; Vo[6] = y[6] * Vo[7]
    Vo[8] = y[8]
    import ml_dtypes
    vt = np.stack([V, Vo], 1)                                   # (9, 2, OC, IC)
    vflat = vt.transpose(3, 0, 1, 2).reshape(IC, VFREE)
    vtab = np.ascontiguousarray(
        np.tile(vflat, (RG, 1)).astype(ml_dtypes.bfloat16))     # (128, 1152)

    xpads = []
    for i in range(x.shape[0]):
        xpi = np.pad(x[i], ((0, 0), (1, 1), (1, 1)))
        planes = np.zeros((RG, IC, 2, PADR, PLW), np.float32)
        for rg in range(RG):
            blk = xpi[:, 16 * rg:16 * rg + PADR, :]        # (32, 18, 66)
            for q in range(2):
                for r in range(PADR):
                    c0 = (r + q) % 2
                    planes[rg, :, q, r, 0:33] = blk[:, r, c0::2]
        xpads.append(np.ascontiguousarray(
            planes.reshape(128, XFREE).astype(ml_dtypes.bfloat16)))
    return xpads, vtab


_RGI = np.arange(RG)[:, None, None, None]
_SI = np.arange(8)[None, :, None, None]
_EI = np.arange(2)[None, None, :, None]
_MI = np.arange(32)[None, None, None, :]
_ROWS = 16 * _RGI + 2 * _SI + _EI
_COLE = 2 * _MI + _EI
_COLO = 2 * _MI + 1 - _EI


def decode_out(out):
    o = out.reshape(2, OC, RG, 8, 2, 32)
    full = np.empty((OC, H, W), np.float32)
    full[:, _ROWS, _COLE] = o[0]
    full[:, _ROWS, _COLO] = o[1]
    return full


def kernel(x, weight):
    _, _, _, bass_utils = _bass_modules()
    x = np.ascontiguousarray(np.asarray(x), dtype=np.float32)
    weight = np.ascontiguousarray(np.asarray(weight), dtype=np.float32)
    xpads, vtab = host_inputs(x, weight)
    nc = build_nc()
    in_maps = [{"xpad": xpads[i], "vtab": vtab} for i in range(N)]
    res = bass_utils.run_bass_kernel_spmd(nc, in_maps, core_ids=list(range(N)))
    return np.stack([decode_out(res.results[i]["out"]) for i in range(N)], 0)


# revision 37
# speedup vs baseline: 1.1241x; 1.0432x over previous
"""AEG-Conv2d Trainium2 kernel (8 NeuronCores, data-parallel over batch).

Math: the reference's 9-step scan  r <- (r+x)*y / (r+y)*x  (parity of i+j+k)
unrolls to  r = sum_k a_k * prod_{j>=k} m_j, which factors per output-pixel
parity into  out[n,oc,px] = sum_{t=0..8} sum_ic U_t[n,ic,px] * V_t[oc,ic]:
a 288-deep contraction where U_t are products of shifted input patches
(computed on-chip) and V_t are products of weight taps (computed on host).

Per-core layout: partition p = rg*32 + ic (rg = 4 row-groups of 16 output
rows, with 18-row halo of the 66-wide padded image in the free dim).
Checkerboard pixel sets are addressed with diagonal access patterns
[(132,8),(67|65,2),(2,32)] giving dense 512-px per-parity feature tiles.
Matmuls are K=32 (ic) row-tiled at partition base 32*rg, M=64 (oc),
accumulating 9 taps into PSUM bank rg ([0:64]=even-parity out, [64:128]=odd).
"""

import numpy as np

IC, OC, H, W = 32, 64, 64, 64
N = 8
RG = 4           # row groups per core
PADW, PADR = 66, 18
PLW = 34         # parity-plane row width (33 entries + 1 pad)
PLSZ = PADR * PLW            # 612 per plane
XFREE = 2 * PLSZ             # 1224
VFREE = 9 * 2 * OC           # 1152
OUTFREE = RG * 512           # 2048

MM_DTYPE = "bfloat16"        # matmul input dtype (1 PE cycle/row; fp32 would be 4)
N_WARM = 7                   # dummy full-mode matmuls to warm the PE clock

# feature chains: (t, factor_tap, source_t); U_t = P_factor ⊙ U_source.
# sources 7,8 (even) and 8 (odd) are raw f32 patch views into xp.
# Split across engines so DVE and GPSIMD produce features concurrently
# (GPSIMD TT is ~2x slower per op, so it gets the short odd-parity spine).
# all feature muls on DVE: concurrent GPSIMD tensor ops slow DVE ~3x
# (port sharing), so a single 2x-mode DVE stream is fastest.
_DVE_OPS = [  # (parity, t, factor, src)
    (0, 5, 5, 7), (0, 6, 6, 7), (1, 7, 7, 8), (1, 6, 6, 8),
    (0, 3, 3, 5), (0, 4, 4, 5), (1, 5, 5, 6), (1, 4, 4, 6),
    (0, 1, 1, 3), (0, 2, 2, 3), (1, 3, 3, 4), (1, 2, 2, 4),
    (0, 0, 0, 1), (1, 1, 1, 2), (1, 0, 0, 2),
]
_GP_OPS = []
_ROOTS = [(0, 7), (0, 8), (1, 8)]       # raw plane views (no materialization)
_MM_ORDER = {0: [7, 8, 5, 6, 3, 4, 1, 2, 0], 1: [8, 7, 6, 5, 4, 3, 2, 1, 0]}


def _bass_modules():
    import sys
    try:
        import concourse.bass as bass
    except ImportError:
        sys.path.insert(0, "/opt/trn_rl_repo")
        import concourse.bass as bass
    import concourse.mybir as mybir
    import concourse.tile as tile
    from concourse import bass_utils
    return bass, mybir, tile, bass_utils


def _hoist_input_dmas(nc, mybir, names):
    """Move the input-load DMACopy triggers (no waits, SP engine) from the
    body block into the prologue block, before SP enters the all-engine
    barrier — the loads then overlap the other engines' preamble."""
    f = nc.m.functions[0]
    blocks = list(f.blocks)
    pro, body = blocks[0], blocks[1]
    moved = []
    bil = body.instructions
    i = 0
    while i < len(bil):
        inst = bil[i]
        si = inst.sync_info
        if (type(inst).__name__ == "InstDMACopy"
                and (si is None or not list(si.on_wait))):
            moved.append(inst)    # input loads are the only wait-free DMAs
            del bil[i]
            continue
        i += 1
    pil = pro.instructions
    # insert before the SP barrier drain (first SP InstDrain in prologue)
    idx = 0
    for j, inst in enumerate(pil):
        if type(inst).__name__ == "InstDrain" and inst.engine == mybir.EngineType.SP:
            idx = j
            break
    for k, inst in enumerate(moved):
        pil.insert(idx + k, inst)


def _split_multi_waits(nc, mybir, limit=1):
    """walrus codegen in this toolchain allows only one sync-wait command per
    engine instruction; hoist surplus waits into standalone InstEventSemaphore
    instructions inserted just before, on the same engine queue."""
    n = 0
    for f in nc.m.functions:
        for b in f.blocks:
            il = b.instructions
            i = 0
            while i < len(il):
                inst = il[i]
                si = inst.sync_info
                if si is not None and len(si.on_wait) > limit:
                    waits = list(si.on_wait)
                    for w in waits[:-limit]:
                        ev = mybir.InstEventSemaphore(
                            name=f"wsplit_{n}", ins=[], outs=[])
                        n += 1
                        ev.engine = inst.engine
                        ev.sync_info = mybir.SyncInfo(on_wait=[w], on_update=[])
                        il.insert(i, ev)
                        i += 1
                    inst.sync_info = mybir.SyncInfo(
                        on_wait=waits[-limit:], on_update=list(si.on_update))
                i += 1


def build_nc(split_waits=True):
    bass, mybir, tile, _ = _bass_modules()
    F32 = mybir.dt.float32
    mmdt = getattr(mybir.dt, MM_DTYPE)
    nc = bass.Bass()
    x_in = nc.declare_dram_parameter("xpad", [128, XFREE], mybir.dt.bfloat16,
                                     isOutput=False)
    v_in = nc.declare_dram_parameter("vtab", [128, VFREE], mmdt, isOutput=False)
    out_ext = nc.declare_dram_parameter("out", [128, OUTFREE], F32, isOutput=True)
    BF16 = mybir.dt.bfloat16

    def diag(xp_ap, k, par, rg=None):
        """Patch-tap view over the parity-plane xp layout.
        Free dims (s,e,m) address output px (row 2s+e, col 2m+((par+e)%2));
        tap k=(dh,dw) reads plane (par+dh+dw)%2 densely (inner step 1)."""
        dh, dw = divmod(k, 3)
        qk = (par + dh + dw) % 2
        def delta(e):
            return ((par + e) % 2 + dw - (e + dh + qk) % 2) // 2
        base = qk * PLSZ + dh * PLW + delta(0)
        estep = PLW + delta(1) - delta(0)
        v = xp_ap.copy()
        pstep = tuple(list(v.ap)[0])[0]
        poff = 0 if rg is None else 32 * rg * pstep
        cnt = 128 if rg is None else 32
        v.ap = mybir.VecI64Pair([(pstep, cnt), (2 * PLW, 8), (estep, 2), (1, 32)])
        v.offset = v.offset + poff + base
        return v

    with tile.TileContext(nc) as tc:
        with tc.tile_pool(name="sb", bufs=1) as pool, \
             tc.tile_pool(name="ps", bufs=1, space="PSUM") as pp:
            xp = pool.tile([128, XFREE], BF16, tag="xp")
            nc.sync.dma_start(xp[:], x_in[:])
            vt = pool.tile([128, VFREE], mmdt, tag="vt")
            nc.sync.dma_start(vt[:], v_in[:])
            outb = pool.tile([128, OUTFREE], F32, tag="outb")

            feats = {}

            def newfeat(par, t):
                ft = pool.tile([128, 512], mmdt, tag=f"f{par}_{t}", name=f"f{par}_{t}")
                feats[(par, t)] = ft
                return ft[:].rearrange("p (s e m) -> p s e m", e=2, m=32)

            rootset = {(p, t) for p, t in _ROOTS}

            def src_ap(par, src):
                if (par, src) in rootset:
                    return diag(xp[:], src, par)   # raw plane view, no copy
                return feats[(par, src)][:].rearrange(
                    "p (s e m) -> p s e m", e=2, m=32)

            # chain muls, interleaved across DVE and GPSIMD
            qs = {0: list(_DVE_OPS), 1: list(_GP_OPS)}
            engines = {0: nc.vector, 1: nc.gpsimd}
            while qs[0] or qs[1]:
                for q in (0, 0, 1):   # ~3 DVE ops per GP op
                    if qs[q]:
                        par, t, kf, src = qs[q].pop(0)
                        in1 = src_ap(par, src)
                        engines[q].tensor_mul(
                            newfeat(par, t), diag(xp[:], kf, par), in1)

            psums = [pp.tile([128, 512], F32, tag=f"acc{rg}", name=f"acc{rg}")
                     for rg in range(RG)]
            # PE warm-up: tiled-mode (K=32) matmuls don't engage the HAM
            # clock gate, leaving the PE at 1.2 GHz. Run full-width dummy
            # matmuls on the vt tile while DVE/GPSIMD produce features, so
            # the real accumulations run at 2.4 GHz.
            wps = pp.tile([128, 512], F32, tag="warm", name="warm")
            for w in range(N_WARM):
                nc.tensor.matmul(wps[:], lhsT=vt[:, 0:128], rhs=vt[:, 0:512],
                                 start=True, stop=True, skip_group_check=True)
            for i in range(9):
                # keep the HAM clock-gate open: tiled MMs don't count as
                # PE-busy, so feed it one full-mode MM per round
                nc.tensor.matmul(wps[:], lhsT=vt[:, 0:128], rhs=vt[:, 0:512],
                                 start=True, stop=True, skip_group_check=True)
                for par in (0, 1):
                    t = _MM_ORDER[par][i]
                    for rg in range(RG):
                        if (par, t) in feats:
                            rhs = feats[(par, t)][32 * rg:32 * (rg + 1), :]
                        else:
                            rhs = diag(xp[:], t, par, rg=rg)  # raw plane view
                        lhsT = vt[32 * rg:32 * (rg + 1),
                                  t * 128 + 64 * par: t * 128 + 64 * par + 64]
                        nc.tensor.matmul(
                            psums[rg][64 * par:64 * par + 64, :],
                            lhsT=lhsT, rhs=rhs,
                            start=(i == 0), stop=(i == 8),
                            skip_group_check=True,
                            tile_position=(32 * rg, 64 * par))
            # evacuate PSUM on both ScalarE and VectorE; stream each bank's
            # result to DRAM as soon as it is copied
            for rg in range(RG):
                dst = outb[:, 512 * rg:512 * (rg + 1)]
                if rg % 2 == 0:
                    nc.scalar.copy(dst, psums[rg][:])
                else:
                    nc.vector.tensor_copy(dst, psums[rg][:])
                nc.sync.dma_start(
                    out_ext[:, 512 * rg:512 * (rg + 1)], dst)
    if split_waits:
        _hoist_input_dmas(nc, mybir, ("xpad", "vtab"))
        _split_multi_waits(nc, mybir)
    return nc


def host_inputs(x, weight):
    y = weight.reshape(OC, IC, 9).transpose(2, 0, 1).astype(np.float64)
    V = np.empty_like(y)
    V[8] = y[8]; V[6] = y[6] * V[8]; V[4] = y[4] * V[6]; V[2] = y[2] * V[4]
    V[0] = y[0] * V[2]; V[1] = y[1] * V[2]; V[3] = y[3] * V[4]; V[5] = y[5] * V[6]
    V[7] = y[7] * V[8]
    Vo = np.empty_like(y)
    Vo[7] = y[7]; Vo[5] = y[5] * Vo[7]; Vo[3] = y[3] * Vo[5]; Vo[1] = y[1] * Vo[3]
    Vo[0] = y[0] * Vo[1]; Vo[2] = y[2] * Vo[3]; Vo[4] = y[4] * Vo[5]; Vo[6] = y[6] * Vo[7]
    Vo[8] = y[8]
    import ml_dtypes
    vt = np.stack([V, Vo], 1)                                   # (9, 2, OC, IC)
    vflat = vt.transpose(3, 0, 1, 2).reshape(IC, VFREE)
    vtab = np.ascontiguousarray(
        np.tile(vflat, (RG, 1)).astype(ml_dtypes.bfloat16))     # (128, 1152)

    xpads = []
    for i in range(x.shape[0]):
        xpi = np.pad(x[i], ((0, 0), (1, 1), (1, 1)))
        planes = np.zeros((RG, IC, 2, PADR, PLW), np.float32)
        for rg in range(RG):
            blk = xpi[:, 16 * rg:16 * rg + PADR, :]        # (32, 18, 66)
            for q in range(2):
                for r in range(PADR):
                    c0 = (r + q) % 2
                    planes[rg, :, q, r, 0:33] = blk[:, r, c0::2]
        xpads.append(np.ascontiguousarray(
            planes.reshape(128, XFREE).astype(ml_dtypes.bfloat16)))
    return xpads, vtab


_RGI = np.arange(RG)[:, None, None, None]
_SI = np.arange(8)[None, :, None, None]
_EI = np.arange(2)[None, None, :, None]
_MI = np.arange(32)[None, None, None, :]
_ROWS = 16 * _RGI + 2 * _SI + _EI
_COLE = 2 * _MI + _EI
_COLO = 2 * _MI + 1 - _EI


def decode_out(out):
    o = out.reshape(2, OC, RG, 8, 2, 32)
    full = np.empty((OC, H, W), np.float32)
    full[:, _ROWS, _COLE] = o[0]
    full[:, _ROWS, _COLO] = o[1]
    return full


def kernel(x, weight):
    _, _, _, bass_utils = _bass_modules()
    x = np.ascontiguousarray(np.asarray(x), dtype=np.float32)
    weight = np.ascontiguousarray(np.asarray(weight), dtype=np.float32)
    xpads, vtab = host_inputs(x, weight)
    nc = build_nc()
    in_maps = [{"xpad": xpads[i], "vtab": vtab} for i in range(N)]
    res = bass_utils.run_bass_kernel_spmd(nc, in_maps, core_ids=list(range(N)))
    return np.stack([decode_out(res.results[i]["out"]) for i in range(N)], 0)


# revision 39
# speedup vs baseline: 1.2437x; 1.1064x over previous
"""AEG-Conv2d Trainium2 kernel (8 NeuronCores, data-parallel over batch).

Math: the reference's 9-step scan  r <- (r+x)*y / (r+y)*x  (parity of i+j+k)
unrolls to  r = sum_k a_k * prod_{j>=k} m_j, which factors per output-pixel
parity into  out[n,oc,px] = sum_{t=0..8} sum_ic U_t[n,ic,px] * V_t[oc,ic]:
a 288-deep contraction where U_t are products of shifted input patches
(computed on-chip) and V_t are products of weight taps (computed on host).

Per-core layout: partition p = rg*32 + ic (rg = 4 row-groups of 16 output
rows, with 18-row halo of the 66-wide padded image in the free dim).
Checkerboard pixel sets are addressed with diagonal access patterns
[(132,8),(67|65,2),(2,32)] giving dense 512-px per-parity feature tiles.
Matmuls are K=32 (ic) row-tiled at partition base 32*rg, M=64 (oc),
accumulating 9 taps into PSUM bank rg ([0:64]=even-parity out, [64:128]=odd).
"""

import numpy as np

IC, OC, H, W = 32, 64, 64, 64
N = 8
RG = 4           # row groups per core
PADW, PADR = 66, 18
PLW = 34         # parity-plane row width (33 entries + 1 pad)
PLSZ = PADR * PLW            # 612 per plane
XFREE = 2 * PLSZ             # 1224
VFREE = 9 * 2 * OC           # 1152
OUTFREE = RG * 512           # 2048

MM_DTYPE = "bfloat16"        # matmul input dtype (1 PE cycle/row; fp32 would be 4)
N_WARM = 7                   # dummy full-mode matmuls to warm the PE clock

# feature chains: (t, factor_tap, source_t); U_t = P_factor ⊙ U_source.
# sources 7,8 (even) and 8 (odd) are raw f32 patch views into xp.
# Split across engines so DVE and GPSIMD produce features concurrently
# (GPSIMD TT is ~2x slower per op, so it gets the short odd-parity spine).
# all feature muls on DVE: concurrent GPSIMD tensor ops slow DVE ~3x
# (port sharing), so a single 2x-mode DVE stream is fastest.
_DVE_OPS = [  # (parity, t, factor, src)
    (0, 5, 5, 7), (0, 6, 6, 7), (1, 7, 7, 8), (1, 6, 6, 8),
    (0, 3, 3, 5), (0, 4, 4, 5), (1, 5, 5, 6), (1, 4, 4, 6),
    (0, 1, 1, 3), (0, 2, 2, 3), (1, 3, 3, 4), (1, 2, 2, 4),
    (0, 0, 0, 1), (1, 1, 1, 2), (1, 0, 0, 2),
]
_GP_OPS = []
_ROOTS = [(0, 7), (0, 8), (1, 8)]       # raw plane views (no materialization)
_MM_ORDER = {0: [7, 8, 5, 6, 3, 4, 1, 2, 0], 1: [8, 7, 6, 5, 4, 3, 2, 1, 0]}


def _bass_modules():
    import sys
    try:
        import concourse.bass as bass
    except ImportError:
        sys.path.insert(0, "/opt/trn_rl_repo")
        import concourse.bass as bass
    import concourse.mybir as mybir
    import concourse.tile as tile
    from concourse import bass_utils
    return bass, mybir, tile, bass_utils


def _hoist_input_dmas(nc, mybir, names):
    """Move the input-load DMACopy triggers (no waits, SP engine) from the
    body block into the prologue block, before SP enters the all-engine
    barrier — the loads then overlap the other engines' preamble."""
    f = nc.m.functions[0]
    blocks = list(f.blocks)
    pro, body = blocks[0], blocks[1]
    moved = []
    bil = body.instructions
    i = 0
    while i < len(bil):
        inst = bil[i]
        si = inst.sync_info
        if (type(inst).__name__ == "InstDMACopy"
                and (si is None or not list(si.on_wait))):
            moved.append(inst)    # input loads are the only wait-free DMAs
            del bil[i]
            continue
        i += 1
    pil = pro.instructions
    # insert before the SP barrier drain (first SP InstDrain in prologue)
    idx = 0
    for j, inst in enumerate(pil):
        if type(inst).__name__ == "InstDrain" and inst.engine == mybir.EngineType.SP:
            idx = j
            break
    for k, inst in enumerate(moved):
        pil.insert(idx + k, inst)


def _split_multi_waits(nc, mybir, limit=1):
    """walrus codegen in this toolchain allows only one sync-wait command per
    engine instruction; hoist surplus waits into standalone InstEventSemaphore
    instructions inserted just before, on the same engine queue."""
    n = 0
    for f in nc.m.functions:
        for b in f.blocks:
            il = b.instructions
            i = 0
            while i < len(il):
                inst = il[i]
                si = inst.sync_info
                if si is not None and len(si.on_wait) > limit:
                    waits = list(si.on_wait)
                    for w in waits[:-limit]:
                        ev = mybir.InstEventSemaphore(
                            name=f"wsplit_{n}", ins=[], outs=[])
                        n += 1
                        ev.engine = inst.engine
                        ev.sync_info = mybir.SyncInfo(on_wait=[w], on_update=[])
                        il.insert(i, ev)
                        i += 1
                    inst.sync_info = mybir.SyncInfo(
                        on_wait=waits[-limit:], on_update=list(si.on_update))
                i += 1


def build_nc(split_waits=True):
    bass, mybir, tile, _ = _bass_modules()
    F32 = mybir.dt.float32
    mmdt = getattr(mybir.dt, MM_DTYPE)
    nc = bass.Bass()
    x_in = nc.declare_dram_parameter("xpad", [128, XFREE], mybir.dt.bfloat16,
                                     isOutput=False)
    v_in = nc.declare_dram_parameter("vtab", [128, VFREE], mmdt, isOutput=False)
    out_ext = nc.declare_dram_parameter("out", [128, OUTFREE], F32, isOutput=True)
    BF16 = mybir.dt.bfloat16

    def diag(xp_ap, k, par, rg=None):
        """Patch-tap view over the parity-plane xp layout.
        Free dims (s,e,m) address output px (row 2s+e, col 2m+((par+e)%2));
        tap k=(dh,dw) reads plane (par+dh+dw)%2 densely (inner step 1)."""
        dh, dw = divmod(k, 3)
        qk = (par + dh + dw) % 2
        def delta(e):
            return ((par + e) % 2 + dw - (e + dh + qk) % 2) // 2
        base = qk * PLSZ + dh * PLW + delta(0)
        estep = PLW + delta(1) - delta(0)
        v = xp_ap.copy()
        pstep = tuple(list(v.ap)[0])[0]
        poff = 0 if rg is None else 32 * rg * pstep
        cnt = 128 if rg is None else 32
        v.ap = mybir.VecI64Pair([(pstep, cnt), (2 * PLW, 8), (estep, 2), (1, 32)])
        v.offset = v.offset + poff + base
        return v

    with tile.TileContext(nc) as tc:
        with tc.tile_pool(name="sb", bufs=1) as pool, \
             tc.tile_pool(name="ps", bufs=1, space="PSUM") as pp:
            xp = pool.tile([128, XFREE], BF16, tag="xp")
            nc.sync.dma_start(xp[:], x_in[:])
            vt = pool.tile([128, VFREE], mmdt, tag="vt")
            nc.sync.dma_start(vt[:], v_in[:])
            outb = pool.tile([128, OUTFREE], F32, tag="outb")

            feats = {}

            def newfeat(par, t):
                ft = pool.tile([128, 512], mmdt, tag=f"f{par}_{t}", name=f"f{par}_{t}")
                feats[(par, t)] = ft
                return ft[:].rearrange("p (s e m) -> p s e m", e=2, m=32)

            rootset = {(p, t) for p, t in _ROOTS}

            def src_ap(par, src):
                if (par, src) in rootset:
                    return diag(xp[:], src, par)   # raw plane view, no copy
                return feats[(par, src)][:].rearrange(
                    "p (s e m) -> p s e m", e=2, m=32)

            # chain muls, interleaved across DVE and GPSIMD
            qs = {0: list(_DVE_OPS), 1: list(_GP_OPS)}
            engines = {0: nc.vector, 1: nc.gpsimd}
            while qs[0] or qs[1]:
                for q in (0, 0, 1):   # ~3 DVE ops per GP op
                    if qs[q]:
                        par, t, kf, src = qs[q].pop(0)
                        in1 = src_ap(par, src)
                        engines[q].tensor_mul(
                            newfeat(par, t), diag(xp[:], kf, par), in1)

            psums = [pp.tile([128, 512], F32, tag=f"acc{rg}", name=f"acc{rg}")
                     for rg in range(RG)]
            # PE warm-up: tiled-mode (K=32) matmuls don't engage the HAM
            # clock gate, leaving the PE at 1.2 GHz. Run full-width dummy
            # matmuls on uninitialized SBUF (no input dependency, so they
            # start during the other engines' preamble) to open the gate
            # before the real accumulations.
            wps = pp.tile([128, 512], F32, tag="warm", name="warm")
            junk = outb[:].bitcast(BF16)
            for w in range(N_WARM):
                nc.tensor.matmul(wps[:], lhsT=junk[:, 0:128], rhs=junk[:, 512:1024],
                                 start=True, stop=True, skip_group_check=True)
            for i in range(9):
                # keep the HAM clock-gate open: tiled MMs don't count as
                # PE-busy, so feed it a full-mode MM every few rounds
                if i % 4 == 3:
                    nc.tensor.matmul(wps[:], lhsT=junk[:, 0:128],
                                     rhs=junk[:, 512:1024],
                                     start=True, stop=True, skip_group_check=True)
                for par in (0, 1):
                    t = _MM_ORDER[par][i]
                    for rg in range(RG):
                        if (par, t) in feats:
                            rhs = feats[(par, t)][32 * rg:32 * (rg + 1), :]
                        else:
                            rhs = diag(xp[:], t, par, rg=rg)  # raw plane view
                        lhsT = vt[32 * rg:32 * (rg + 1),
                                  t * 128 + 64 * par: t * 128 + 64 * par + 64]
                        nc.tensor.matmul(
                            psums[rg][64 * par:64 * par + 64, :],
                            lhsT=lhsT, rhs=rhs,
                            start=(i == 0), stop=(i == 8),
                            skip_group_check=True,
                            tile_position=(32 * rg, 64 * par))
            # evacuate PSUM on both ScalarE and VectorE; stream each bank's
            # result to DRAM as soon as it is copied
            for rg in range(RG):
                dst = outb[:, 512 * rg:512 * (rg + 1)]
                if rg % 2 == 0:
                    nc.scalar.copy(dst, psums[rg][:])
                else:
                    nc.vector.tensor_copy(dst, psums[rg][:])
                nc.sync.dma_start(
                    out_ext[:, 512 * rg:512 * (rg + 1)], dst)
    if split_waits:
        _hoist_input_dmas(nc, mybir, ("xpad", "vtab"))
        _split_multi_waits(nc, mybir)
    return nc


def host_inputs(x, weight):
    y = weight.reshape(OC, IC, 9).transpose(2, 0, 1).astype(np.float64)
    V = np.empty_like(y)
    V[8] = y[8]; V[6] = y[6] * V[8]; V[4] = y[4] * V[6]; V[2] = y[2] * V[4]
    V[0] = y[0] * V[2]; V[1] = y[1] * V[2]; V[3] = y[3] * V[4]; V[5] = y[5] * V[6]
    V[7] = y[7] * V[8]
    Vo = np.empty_like(y)
    Vo[7] = y[7]; Vo[5] = y[5] * Vo[7]; Vo[3] = y[3] * Vo[5]; Vo[1] = y[1] * Vo[3]
    Vo[0] = y[0] * Vo[1]; Vo[2] = y[2] * Vo[3]; Vo[4] = y[4] * Vo[5]; Vo[6] = y[6] * Vo[7]
    Vo[8] = y[8]
    import ml_dtypes
    vt = np.stack([V, Vo], 1)                                   # (9, 2, OC, IC)
    vflat = vt.transpose(3, 0, 1, 2).reshape(IC, VFREE)
    vtab = np.ascontiguousarray(
        np.tile(vflat, (RG, 1)).astype(ml_dtypes.bfloat16))     # (128, 1152)

    xpads = []
    for i in range(x.shape[0]):
        xpi = np.pad(x[i], ((0, 0), (1, 1), (1, 1)))
        planes = np.zeros((RG, IC, 2, PADR, PLW), np.float32)
        for rg in range(RG):
            blk = xpi[:, 16 * rg:16 * rg + PADR, :]        # (32, 18, 66)
            for q in range(2):
                for r in range(PADR):
                    c0 = (r + q) % 2
                    planes[rg, :, q, r, 0:33] = blk[:, r, c0::2]
        xpads.append(np.ascontiguousarray(
            planes.reshape(128, XFREE).astype(ml_dtypes.bfloat16)))
    return xpads, vtab


_RGI = np.arange(RG)[:, None, None, None]
_SI = np.arange(8)[None, :, None, None]
_EI = np.arange(2)[None, None, :, None]
_MI = np.arange(32)[None, None, None, :]
_ROWS = 16 * _RGI + 2 * _SI + _EI
_COLE = 2 * _MI + _EI
_COLO = 2 * _MI + 1 - _EI


def decode_out(out):
    o = out.reshape(2, OC, RG, 8, 2, 32)
    full = np.empty((OC, H, W), np.float32)
    full[:, _ROWS, _COLE] = o[0]
    full[:, _ROWS, _COLO] = o[1]
    return full


def kernel(x, weight):
    _, _, _, bass_utils = _bass_modules()
    x = np.ascontiguousarray(np.asarray(x), dtype=np.float32)
    weight = np.ascontiguousarray(np.asarray(weight), dtype=np.float32)
    xpads, vtab = host_inputs(x, weight)
    nc = build_nc()
    in_maps = [{"xpad": xpads[i], "vtab": vtab} for i in range(N)]
    res = bass_utils.run_bass_kernel_spmd(nc, in_maps, core_ids=list(range(N)))
    return np.stack([decode_out(res.results[i]["out"]) for i in range(N)], 0)


# revision 42
# speedup vs baseline: 1.3698x; 1.1014x over previous
"""AEG-Conv2d Trainium2 kernel (8 NeuronCores, data-parallel over batch).

Math: the reference's 9-step scan  r <- (r+x)*y / (r+y)*x  (parity of i+j+k)
unrolls to  r = sum_k a_k * prod_{j>=k} m_j, which factors per output-pixel
parity into  out[n,oc,px] = sum_{t=0..8} sum_ic U_t[n,ic,px] * V_t[oc,ic]:
a 288-deep contraction where U_t are products of shifted input patches
(computed on-chip) and V_t are products of weight taps (computed on host).

Per-core layout: partition p = rg*32 + ic (rg = 4 row-groups of 16 output
rows, with 18-row halo of the 66-wide padded image in the free dim).
Checkerboard pixel sets are addressed with diagonal access patterns
[(132,8),(67|65,2),(2,32)] giving dense 512-px per-parity feature tiles.
Matmuls are K=32 (ic) row-tiled at partition base 32*rg, M=64 (oc),
accumulating 9 taps into PSUM bank rg ([0:64]=even-parity out, [64:128]=odd).
"""

import numpy as np

IC, OC, H, W = 32, 64, 64, 64
N = 8
RG = 4           # row groups per core
PADW, PADR = 66, 18
PLW = 34         # parity-plane row width (33 entries + 1 pad)
PLSZ = PADR * PLW            # 612 per plane
XFREE = 2 * PLSZ             # 1224
VFREE = 9 * 2 * OC           # 1152
OUTFREE = RG * 512           # 2048

MM_DTYPE = "bfloat16"        # matmul input dtype (1 PE cycle/row; fp32 would be 4)
N_WARM = 6                   # dummy full-mode matmuls to warm the PE clock

# feature chains: (t, factor_tap, source_t); U_t = P_factor ⊙ U_source.
# sources 7,8 (even) and 8 (odd) are raw f32 patch views into xp.
# Split across engines so DVE and GPSIMD produce features concurrently
# (GPSIMD TT is ~2x slower per op, so it gets the short odd-parity spine).
# all feature muls on DVE: concurrent GPSIMD tensor ops slow DVE ~3x
# (port sharing), so a single 2x-mode DVE stream is fastest.
_DVE_OPS = [  # (parity, t, factor, src)
    (0, 5, 5, 7), (0, 6, 6, 7), (1, 7, 7, 8), (1, 6, 6, 8),
    (0, 3, 3, 5), (0, 4, 4, 5), (1, 5, 5, 6), (1, 4, 4, 6),
    (0, 1, 1, 3), (0, 2, 2, 3), (1, 3, 3, 4), (1, 2, 2, 4),
    (0, 0, 0, 1), (1, 1, 1, 2), (1, 0, 0, 2),
]
_GP_OPS = []
_ROOTS = [(0, 7), (0, 8), (1, 8)]       # raw plane views (no materialization)
_MM_ORDER = {0: [7, 8, 5, 6, 3, 4, 1, 2, 0], 1: [8, 7, 6, 5, 4, 3, 2, 1, 0]}


def _bass_modules():
    import sys
    try:
        import concourse.bass as bass
    except ImportError:
        sys.path.insert(0, "/opt/trn_rl_repo")
        import concourse.bass as bass
    import concourse.mybir as mybir
    import concourse.tile as tile
    from concourse import bass_utils
    return bass, mybir, tile, bass_utils


def _hoist_input_dmas(nc, mybir, names):
    """Move the input-load DMACopy triggers (no waits, SP engine) from the
    body block into the prologue block, before SP enters the all-engine
    barrier — the loads then overlap the other engines' preamble."""
    f = nc.m.functions[0]
    blocks = list(f.blocks)
    pro, body = blocks[0], blocks[1]
    moved = []
    bil = body.instructions
    i = 0
    while i < len(bil):
        inst = bil[i]
        si = inst.sync_info
        if (type(inst).__name__ == "InstDMACopy"
                and (si is None or not list(si.on_wait))):
            moved.append(inst)    # input loads are the only wait-free DMAs
            del bil[i]
            continue
        i += 1
    pil = pro.instructions
    # insert before the SP barrier drain (first SP InstDrain in prologue)
    idx = 0
    for j, inst in enumerate(pil):
        if type(inst).__name__ == "InstDrain" and inst.engine == mybir.EngineType.SP:
            idx = j
            break
    for k, inst in enumerate(moved):
        pil.insert(idx + k, inst)


def _split_multi_waits(nc, mybir, limit=1):
    """walrus codegen in this toolchain allows only one sync-wait command per
    engine instruction; hoist surplus waits into standalone InstEventSemaphore
    instructions inserted just before, on the same engine queue."""
    eng_of_sem = {
        "PE_": mybir.EngineType.PE, "DVE_": mybir.EngineType.DVE,
        "Activation_": mybir.EngineType.Activation,
        "Pool_": mybir.EngineType.Pool,
    }
    n = 0
    for f in nc.m.functions:
        for b in f.blocks:
            il = b.instructions
            i = 0
            while i < len(il):
                inst = il[i]
                si = inst.sync_info
                if si is not None and len(si.on_wait) > limit:
                    waits = list(si.on_wait)
                    for w in waits[:-limit]:
                        ev = mybir.InstEventSemaphore(
                            name=f"wsplit_{n}", ins=[], outs=[])
                        n += 1
                        # engine-completion waits run in parallel on the
                        # owning engine (pre-barrier) instead of queueing
                        # serially on the instruction's engine
                        ev.engine = inst.engine
                        if type(inst).__name__ == "InstDrain":
                            for pfx, eng in eng_of_sem.items():
                                if w.ant_name.startswith(pfx) and eng != inst.engine:
                                    ev.engine = eng
                                    break
                        ev.sync_info = mybir.SyncInfo(on_wait=[w], on_update=[])
                        il.insert(i, ev)
                        i += 1
                    inst.sync_info = mybir.SyncInfo(
                        on_wait=waits[-limit:], on_update=list(si.on_update))
                i += 1


def build_nc(split_waits=True):
    bass, mybir, tile, _ = _bass_modules()
    F32 = mybir.dt.float32
    mmdt = getattr(mybir.dt, MM_DTYPE)
    nc = bass.Bass()
    x_in = nc.declare_dram_parameter("xpad", [128, XFREE], mybir.dt.bfloat16,
                                     isOutput=False)
    v_in = nc.declare_dram_parameter("vtab", [128, VFREE], mmdt, isOutput=False)
    out_ext = nc.declare_dram_parameter("out", [128, OUTFREE], F32, isOutput=True)
    BF16 = mybir.dt.bfloat16

    def diag(xp_ap, k, par, rg=None):
        """Patch-tap view over the parity-plane xp layout.
        Free dims (s,e,m) address output px (row 2s+e, col 2m+((par+e)%2));
        tap k=(dh,dw) reads plane (par+dh+dw)%2 densely (inner step 1)."""
        dh, dw = divmod(k, 3)
        qk = (par + dh + dw) % 2
        def delta(e):
            return ((par + e) % 2 + dw - (e + dh + qk) % 2) // 2
        base = qk * PLSZ + dh * PLW + delta(0)
        estep = PLW + delta(1) - delta(0)
        v = xp_ap.copy()
        pstep = tuple(list(v.ap)[0])[0]
        poff = 0 if rg is None else 32 * rg * pstep
        cnt = 128 if rg is None else 32
        v.ap = mybir.VecI64Pair([(pstep, cnt), (2 * PLW, 8), (estep, 2), (1, 32)])
        v.offset = v.offset + poff + base
        return v

    with tile.TileContext(nc) as tc:
        with tc.tile_pool(name="sb", bufs=1) as pool, \
             tc.tile_pool(name="ps", bufs=1, space="PSUM") as pp:
            xp = pool.tile([128, XFREE], BF16, tag="xp")
            # plane 1 first: the first feature ops read only plane 1
            nc.sync.dma_start(xp[:, PLSZ:XFREE], x_in[:, PLSZ:XFREE])
            nc.sync.dma_start(xp[:, 0:PLSZ], x_in[:, 0:PLSZ])
            vt = pool.tile([128, VFREE], mmdt, tag="vt")
            nc.sync.dma_start(vt[:], v_in[:])
            outb = pool.tile([128, OUTFREE], F32, tag="outb")

            feats = {}

            def newfeat(par, t):
                ft = pool.tile([128, 512], mmdt, tag=f"f{par}_{t}", name=f"f{par}_{t}")
                feats[(par, t)] = ft
                return ft[:].rearrange("p (s e m) -> p s e m", e=2, m=32)

            rootset = {(p, t) for p, t in _ROOTS}

            def src_ap(par, src):
                if (par, src) in rootset:
                    return diag(xp[:], src, par)   # raw plane view, no copy
                return feats[(par, src)][:].rearrange(
                    "p (s e m) -> p s e m", e=2, m=32)

            # chain muls, interleaved across DVE and GPSIMD
            qs = {0: list(_DVE_OPS), 1: list(_GP_OPS)}
            engines = {0: nc.vector, 1: nc.gpsimd}
            while qs[0] or qs[1]:
                for q in (0, 0, 1):   # ~3 DVE ops per GP op
                    if qs[q]:
                        par, t, kf, src = qs[q].pop(0)
                        in1 = src_ap(par, src)
                        engines[q].tensor_mul(
                            newfeat(par, t), diag(xp[:], kf, par), in1)

            psums = [pp.tile([128, 512], F32, tag=f"acc{rg}", name=f"acc{rg}")
                     for rg in range(RG)]
            # PE warm-up: tiled-mode (K=32) matmuls don't engage the HAM
            # clock gate, leaving the PE at 1.2 GHz. Run full-width dummy
            # matmuls on uninitialized SBUF (no input dependency, so they
            # start during the other engines' preamble) to open the gate
            # before the real accumulations.
            wps = pp.tile([128, 512], F32, tag="warm", name="warm")
            junk = outb[:].bitcast(BF16)
            for w in range(N_WARM):
                nc.tensor.matmul(wps[:], lhsT=junk[:, 0:128], rhs=junk[:, 512:1024],
                                 start=True, stop=True, skip_group_check=True)
            for i in range(9):
                # keep the HAM clock-gate open: tiled MMs don't count as
                # PE-busy, so feed it a full-mode MM every few rounds
                if i % 4 == 3:
                    nc.tensor.matmul(wps[:], lhsT=junk[:, 0:128],
                                     rhs=junk[:, 512:1024],
                                     start=True, stop=True, skip_group_check=True)
                for par in (0, 1):
                    t = _MM_ORDER[par][i]
                    for rg in range(RG):
                        if (par, t) in feats:
                            rhs = feats[(par, t)][32 * rg:32 * (rg + 1), :]
                        else:
                            rhs = diag(xp[:], t, par, rg=rg)  # raw plane view
                        lhsT = vt[32 * rg:32 * (rg + 1),
                                  t * 128 + 64 * par: t * 128 + 64 * par + 64]
                        nc.tensor.matmul(
                            psums[rg][64 * par:64 * par + 64, :],
                            lhsT=lhsT, rhs=rhs,
                            start=(i == 0), stop=(i == 8),
                            skip_group_check=True,
                            tile_position=(32 * rg, 64 * par))
            # evacuate PSUM on both ScalarE and VectorE; stream each bank's
            # result to DRAM as soon as it is copied
            for rg in range(RG):
                dst = outb[:, 512 * rg:512 * (rg + 1)]
                if rg % 2 == 0:
                    nc.scalar.copy(dst, psums[rg][:])
                else:
                    nc.vector.tensor_copy(dst, psums[rg][:])
                nc.sync.dma_start(
                    out_ext[:, 512 * rg:512 * (rg + 1)], dst)
    if split_waits:
        _hoist_input_dmas(nc, mybir, ("xpad", "vtab"))
        _split_multi_waits(nc, mybir)
    return nc


def host_inputs(x, weight):
    y = weight.reshape(OC, IC, 9).transpose(2, 0, 1).astype(np.float64)
    V = np.empty_like(y)
    V[8] = y[8]; V[6] = y[6] * V[8]; V[4] = y[4] * V[6]; V[2] = y[2] * V[4]
    V[0] = y[0] * V[2]; V[1] = y[1] * V[2]; V[3] = y[3] * V[4]; V[5] = y[5] * V[6]
    V[7] = y[7] * V[8]
    Vo = np.empty_like(y)
    Vo[7] = y[7]; Vo[5] = y[5] * Vo[7]; Vo[3] = y[3] * Vo[5]; Vo[1] = y[1] * Vo[3]
    Vo[0] = y[0] * Vo[1]; Vo[2] = y[2] * Vo[3]; Vo[4] = y[4] * Vo[5]; Vo[6] = y[6] * Vo[7]
    Vo[8] = y[8]
    import ml_dtypes
    vt = np.stack([V, Vo], 1)                                   # (9, 2, OC, IC)
    vflat = vt.transpose(3, 0, 1, 2).reshape(IC, VFREE)
    vtab = np.ascontiguousarray(
        np.tile(vflat, (RG, 1)).astype(ml_dtypes.bfloat16))     # (128, 1152)

    xpads = []
    for i in range(x.shape[0]):
        xpi = np.pad(x[i], ((0, 0), (1, 1), (1, 1)))
        planes = np.zeros((RG, IC, 2, PADR, PLW), np.float32)
        for rg in range(RG):
            blk = xpi[:, 16 * rg:16 * rg + PADR, :]        # (32, 18, 66)
            for q in range(2):
                for r in range(PADR):
                    c0 = (r + q) % 2
                    planes[rg, :, q, r, 0:33] = blk[:, r, c0::2]
        xpads.append(np.ascontiguousarray(
            planes.reshape(128, XFREE).astype(ml_dtypes.bfloat16)))
    return xpads, vtab


_RGI = np.arange(RG)[:, None, None, None]
_SI = np.arange(8)[None, :, None, None]
_EI = np.arange(2)[None, None, :, None]
_MI = np.arange(32)[None, None, None, :]
_ROWS = 16 * _RGI + 2 * _SI + _EI
_COLE = 2 * _MI + _EI
_COLO = 2 * _MI + 1 - _EI


def decode_out(out):
    o = out.reshape(2, OC, RG, 8, 2, 32)
    full = np.empty((OC, H, W), np.float32)
    full[:, _ROWS, _COLE] = o[0]
    full[:, _ROWS, _COLO] = o[1]
    return full


def kernel(x, weight):
    _, _, _, bass_utils = _bass_modules()
    x = np.ascontiguousarray(np.asarray(x), dtype=np.float32)
    weight = np.ascontiguousarray(np.asarray(weight), dtype=np.float32)
    xpads, vtab = host_inputs(x, weight)
    nc = build_nc()
    in_maps = [{"xpad": xpads[i], "vtab": vtab} for i in range(N)]
    res = bass_utils.run_bass_kernel_spmd(nc, in_maps, core_ids=list(range(N)))
    return np.stack([decode_out(res.results[i]["out"]) for i in range(N)], 0)
